# revision 45
# baseline (speedup 1.0000x reference)
"""Trainium2 Bass kernel for nn_MultiHeadAttention_26482768347194.

Key algebraic fact: the reference applies softmax over a size-1 trailing
axis, so the attention score matrix is exactly all-ones.  The whole module
collapses (exactly, in real arithmetic) to

    xsum[b]   = sum_l x[b, l, :]                        # (D,)
    t[b]      = xsum[b] @ wv + L * bv                   # (H*D,)
    z[b]      = t[b] @ fc_w + fc_b                      # (D,)
    y[b,l,:]  = x[b,l,:] + z[b]
    out       = LayerNorm(y) * ln_g + ln_b              # over last dim

q/k/tanh/score inputs are mathematically dead.

Sharding: pure data-parallel over batch, one batch element per core,
weights replicated.  Cross-core collectives measured ~70us under this
runtime (launch-skew barrier), so each core runs fully independently:
  1. xsum.T from its x shard (x-stationary PE matmuls against ones),
  2. streamed by 512-wide hd blocks: t.T columns = wv.T @ xsum.T
     (wv chunks stationary, per-column rotating PSUM banks), ACT moves
     each column to SBUF adding L*bv, and matmul2 accumulates
     z = t @ fc_w into a single PSUM bank,
  3. layernorm decomposed so z only enters through cheap terms: with
     zc = (z + fc_b) - mean(z + fc_b),
       var_y[t] = var_x[t] + (2/D) (x_t . zc) + mean(zc^2)
       out[t]   = ((x_t - mean_x[t]) g + zc g) rstd[t] + ln_b
     so per-token x stats and (x-mean_x)*g run on the DVE DURING the
     weight stream, the x.zc dots run on the then-idle PE (against a
     host-supplied x.T), and the post-z critical path is only two DVE
     passes per token tile.

PE matmuls run in bf16 (fp32 is 2-4x slower on the PE and doubles DMA);
statistics and the residual path stay fp32.  End-to-end absmax relative
error vs the fp32 reference: ~3e-3 (bf16 input/weight rounding, well
inside the scale-relative gate).  Measured ~62-66us on hardware.

This file is self-contained: shapes are hardcoded, no sibling imports.
"""

from contextlib import ExitStack

import numpy as np
import ml_dtypes

import concourse.bass as bass
import concourse.bacc as bacc
import concourse.mybir as mybir
import concourse.tile as tile
from concourse.bass_utils import run_bass_kernel_spmd
from concourse.masks import make_identity
from concourse.bass import _add_dep_helper

B, L, D, H = 8, 1024, 512, 8
HD = H * D          # 4096
P = 128             # partitions
NT = L // P         # 8 token tiles per core
KD = D // P         # 4 contraction chunks over d
KO = HD // P        # 32 contraction chunks over h*d
EPS = 1e-5
N_CORES = 8

F32 = mybir.dt.float32
BF16 = mybir.dt.bfloat16
AF = mybir.ActivationFunctionType
ALU = mybir.AluOpType

# which engine runs the final "+ ln_b" pass (see LN phase).  GpSimd shares
# an SBUF port with the DVE (exclusive lock), so offloading there slows
# BOTH engines to ~1.7us per pass — keep it on the DVE.
LN_B_ON_GPSIMD = False


def build_kernel():
    nc = bacc.Bacc("TRN2", target_bir_lowering=False, debug=False,
                   num_devices=N_CORES)

    # x / wv / fc arrive pre-blocked from the host so that every big DMA
    # reads a fully contiguous region:
    #   x[u, p, t, d]  = x_orig[(u*4 + t)*128 + p, d]          (2 x 1MB)
    #   wv[j, p, k, c] = wv_orig[k*128 + p, j*512 + c]         (8 x 1MB)
    #   fc[j, p, q, d] = fc_orig[(j*4 + q)*128 + p, d]         (8 x 1MB)
    #   xT[c, p, l]    = x_orig[l, c*128 + p]                  (4 x 256KB)
    x_d = nc.dram_tensor("x", [2, P, 4, D], BF16, kind="ExternalInput")
    xT_d = nc.dram_tensor("xT", [KD, P, L], BF16, kind="ExternalInput")
    wv_d = nc.dram_tensor("wv", [KO // 4, P, KD, D], BF16, kind="ExternalInput")
    bvT_d = nc.dram_tensor("bvT", [P, KO], F32, kind="ExternalInput")
    fc_d = nc.dram_tensor("fc_w", [KO // 4, P, 4, D], BF16, kind="ExternalInput")
    fcb_d = nc.dram_tensor("fc_b", [1, D], F32, kind="ExternalInput")
    g_d = nc.dram_tensor("ln_g", [1, D], F32, kind="ExternalInput")
    b_d = nc.dram_tensor("ln_b", [1, D], F32, kind="ExternalInput")
    out_d = nc.dram_tensor("out", [L, D], F32, kind="ExternalOutput")

    x_v = x_d.ap()                                               # [2, P, 4, D]
    wv_v = wv_d.ap()                                             # [8, P, KD, D]
    fc_v = fc_d.ap()                                             # [8, P, 4, D]
    out_v = out_d.ap().rearrange("(t p) d -> t p d", p=P)        # [NT, P, D]

    with tile.TileContext(nc) as tc, ExitStack() as ctx:
        consts = ctx.enter_context(tc.tile_pool(name="consts", bufs=1))
        fcp = ctx.enter_context(tc.tile_pool(name="fcp", bufs=1))
        work = ctx.enter_context(tc.tile_pool(name="work", bufs=3))
        ypool = ctx.enter_context(tc.tile_pool(name="ypool", bufs=8))
        psum = ctx.enter_context(
            tc.tile_pool(name="psum", bufs=1, space=bass.MemorySpace.PSUM))

        # ---- constants -------------------------------------------------
        ones_col = consts.tile([P, 1], BF16)     # token-sum matmul rhs
        nc.gpsimd.memset(ones_col[:], 1.0)
        ones2 = consts.tile([2, P], F32)         # z+fc_b broadcast lhsT
        nc.gpsimd.memset(ones2[:], 1.0)
        eps_t = consts.tile([P, 1], F32)
        nc.gpsimd.memset(eps_t[:], EPS)
        ident_bf = consts.tile([1, 1], BF16)   # 1x1 identity for transposes
        nc.gpsimd.memset(ident_bf[:], 1.0)
        ones_row_bf = consts.tile([1, P], BF16)  # broadcast lhsT (bf16)
        nc.gpsimd.memset(ones_row_bf[:], 1.0)
        ident128 = consts.tile([P, P], BF16)     # identity for yg psum pass
        make_identity(nc, ident128)

        bvT_t = consts.tile([P, KO], F32)
        nc.sync.dma_start(bvT_t[:], bvT_d.ap())
        bv1024 = consts.tile([P, KO], F32)
        nc.scalar.mul(bv1024[:], bvT_t[:], float(L))

        fcb_t = consts.tile([1, D], F32)
        nc.sync.dma_start(fcb_t[:], fcb_d.ap())
        g_t = consts.tile([1, D], F32)
        nc.sync.dma_start(g_t[:], g_d.ap())
        b_t = consts.tile([1, D], F32)
        nc.sync.dma_start(b_t[:], b_d.ap())

        # broadcast ln_g / ln_b to all 128 partitions via a K=1 matmul
        g_bc = consts.tile([P, D], F32)
        b_bc = consts.tile([P, D], F32)
        for src, dst in ((g_t, g_bc), (b_t, b_bc)):
            pb = psum.tile([P, D], F32, tag="bcast")
            nc.tensor.matmul(pb[:], ones2[0:1, :], src[:], start=True, stop=True)
            nc.vector.tensor_copy(dst[:], pb[:])

        # ---- load x and x.T (stay resident in SBUF) --------------------
        x_t = consts.tile([P, NT, D], BF16)
        last_x_dma = None
        for u in range(2):
            for h in range(2):
                last_x_dma = nc.sync.dma_start(
                    x_t[:, u * 4 + h * 2:u * 4 + h * 2 + 2, :],
                    x_v[u][:, h * 2:(h + 1) * 2, :])

        # ---- phase A: xsumT[d] = sum over tokens of x ------------------
        # lhsT = x tile slice [128 tok, 128 d], rhs = ones -> psum [128 d, 1]
        ps_xs = psum.tile([P, KD], F32, tag="bcast")
        for c in range(KD):
            for t in range(NT):
                nc.tensor.matmul(
                    ps_xs[:, c:c + 1],
                    x_t[:, t, c * P:(c + 1) * P],
                    ones_col[:],
                    start=(t == 0), stop=(t == NT - 1))
        xsT = consts.tile([P, KD], BF16)
        nc.vector.tensor_copy(xsT[:], ps_xs[:])

        # ---- early layernorm stats on x (overlaps the z phase) ---------
        # Since z is constant across tokens:
        #   y = x + z',  z' = z + fc_b,  mz = mean(z'),  zc = z' - mz
        #   mean_y[t] = mean_x[t] + mz
        #   var_y[t]  = var_x[t] + (2/D) * (x_t . zc) + mean(zc^2)
        #   out[t]    = ((x_t - mean_x[t])*g + zc*g) * rstd[t] + b
        # so per-token x statistics and (x - mean_x)*g run during the z
        # phase on the otherwise-idle DVE, and the x.zc dots run on the
        # otherwise-idle PE afterwards.
        inv_d = 1.0 / D
        varx8 = consts.tile([P, NT], F32)
        xg_tiles = []
        for t in range(NT):
            s6 = work.tile([P, 6], F32, tag="s6")
            nc.vector.bn_stats(s6[:], x_t[:, t, :])
            mv = work.tile([P, 2], F32, tag="mv")
            nc.vector.bn_aggr(mv[:], s6[:])
            nc.vector.tensor_copy(varx8[:, t:t + 1], mv[:, 1:2])
            negmx = work.tile([P, 1], F32, tag="negmx")
            nc.vector.tensor_scalar_mul(negmx[:], mv[:, 0:1], -1.0)
            xg = ypool.tile([P, D], F32, tag="xg")
            nc.vector.scalar_tensor_tensor(
                xg[:], x_t[:, t, :], negmx[:], g_bc[:],
                op0=ALU.add, op1=ALU.mult)
            xg_tiles.append(xg)

        # ---- phase B+C interleaved, streamed by 512-wide hd blocks -----
        # block j: DMA wv cols [4j*128,(4j+4)*128) + fc rows likewise, then
        #   tT col o = sum_kd wv[:, kd, oc*128:..].T @ xsT[:, kd]   (psum)
        #   ACT copies col to SBUF adding L*bv, matmul2 accumulates z.
        tT_sb = consts.tile([P, KO], BF16)
        ps_z = psum.tile([1, D], F32, tag="z")
        for j in range(KO // 4):
            wv_bt = work.tile([P, KD, D], BF16, tag="wvb", bufs=8)
            i_wv = nc.sync.dma_start(wv_bt[:], wv_v[j])
            _add_dep_helper(i_wv.ins, last_x_dma.ins, sync=False,
                            reason="x shard streams before the weight blocks")
            fc_bt = work.tile([P, 4, D], BF16, tag="fcb", bufs=8)
            i_fc = nc.sync.dma_start(fc_bt[:], fc_v[j])
            _add_dep_helper(i_fc.ins, last_x_dma.ins, sync=False,
                            reason="x shard streams before the weight blocks")
            last_w_dma = i_fc
            for oc in range(4):
                o = 4 * j + oc
                # per-column psum tile (rotating banks) so the ACT read of
                # column o doesn't serialize the PE writes of column o+1
                ps_col = psum.tile([P, 1], F32, tag="small", bufs=4)
                for kd in range(KD):
                    nc.tensor.matmul(
                        ps_col[:],
                        wv_bt[:, kd, oc * P:(oc + 1) * P],
                        xsT[:, kd:kd + 1],
                        start=(kd == 0), stop=(kd == KD - 1))
                nc.scalar.activation(tT_sb[:, o:o + 1], ps_col[:],
                                     AF.Identity, bias=bv1024[:, o:o + 1],
                                     scale=1.0)
                nc.tensor.matmul(
                    ps_z[:],
                    tT_sb[:, o:o + 1],
                    fc_bt[:, oc, :],
                    start=(o == 0), stop=(o == KO - 1),
                    skip_group_check=True)

        xT_t = consts.tile([P, KD, L], BF16)
        for c in range(KD):
            i_xt = nc.sync.dma_start(xT_t[:, c, :], xT_d.ap()[c])
            _add_dep_helper(i_xt.ins, last_w_dma.ins, sync=False,
                            reason="xT load deferred behind the weight stream")

        # ---- phase D: z tail -> zc, zg broadcast, variance pieces ------
        zrow = consts.tile([1, D], F32)
        zsum = consts.tile([1, 1], F32)
        nc.vector.scalar_tensor_tensor(
            zrow[:], fcb_t[:], 1.0, ps_z[:], op0=ALU.mult, op1=ALU.add,
            accum_out=zsum[:])
        negmz = consts.tile([1, 1], F32)
        nc.scalar.mul(negmz[:], zsum[:], -inv_d)
        zc = consts.tile([1, D], F32)
        nc.scalar.activation(zc[:], zrow[:], AF.Identity, bias=negmz[:])
        zc_bf = consts.tile([1, D], BF16)
        nc.vector.tensor_copy(zc_bf[:], zc[:])

        # zcT (for the PE dot products)
        zcT = consts.tile([P, KD], BF16)
        for c in range(KD):
            pzt = psum.tile([P, 1], BF16, tag="small", bufs=4)
            nc.tensor.transpose(pzt[:], zc_bf[0:1, c * P:(c + 1) * P],
                                ident_bf[0:1, 0:1])
            nc.vector.tensor_copy(zcT[:, c:c + 1], pzt[:])

        # zg = zc * g, broadcast to 128 partitions
        zg = consts.tile([1, D], F32)
        nc.vector.scalar_tensor_tensor(
            zg[:], zc[:], 1.0, g_t[:], op0=ALU.mult, op1=ALU.mult)
        ps_zg = psum.tile([P, D], F32, tag="bcast")
        nc.tensor.matmul(ps_zg[:], ones2[0:1, :], zg[:], start=True, stop=True)
        zg_bc = consts.tile([P, D], F32)
        nc.vector.tensor_copy(zg_bc[:], ps_zg[:])

        # mean(zc^2) broadcast into a per-partition bias (+ eps)
        zcsq = consts.tile([1, D], F32)
        nc.vector.tensor_mul(zcsq[:], zc[:], zc[:])
        ezsum = consts.tile([1, 1], F32)
        nc.vector.tensor_reduce(ezsum[:], zcsq[:], axis=mybir.AxisListType.X,
                                op=ALU.add)
        ps_ez = psum.tile([P, 1], F32, tag="small", bufs=4)
        nc.tensor.matmul(ps_ez[:], ones2[0:1, :], ezsum[:],
                         start=True, stop=True)
        bias8 = consts.tile([P, 1], F32)
        nc.scalar.activation(bias8[:], ps_ez[:], AF.Identity,
                             bias=eps_t[:], scale=inv_d)

        # ---- phase E: per-token dots on the PE, then two DVE passes ----
        for t in range(NT):
            pd = psum.tile([P, 1], F32, tag="small", bufs=4)
            for c in range(KD):
                nc.tensor.matmul(pd[:], xT_t[:, c, t * P:(t + 1) * P],
                                 zcT[:, c:c + 1],
                                 start=(c == 0), stop=(c == KD - 1))
            var_t = work.tile([P, 1], F32, tag="var_t", bufs=4)
            nc.vector.scalar_tensor_tensor(
                var_t[:], pd[:], 2.0 * inv_d, varx8[:, t:t + 1],
                op0=ALU.mult, op1=ALU.add)
            std_t = work.tile([P, 1], F32, tag="std_t", bufs=4)
            nc.scalar.activation(std_t[:], var_t[:], AF.Sqrt, bias=bias8[:])
            rstd_t = work.tile([P, 1], F32, tag="rstd_t", bufs=4)
            nc.vector.reciprocal(rstd_t[:], std_t[:])

            yg = work.tile([P, D], F32, tag="yg", bufs=4)
            nc.vector.tensor_add(yg[:], xg_tiles[t][:], zg_bc[:])
            o2 = work.tile([P, D], F32, tag="o2", bufs=4)
            nc.vector.scalar_tensor_tensor(
                o2[:], yg[:], rstd_t[:], b_bc[:],
                op0=ALU.mult, op1=ALU.add)
            nc.sync.dma_start(out_v[t], o2[:])

    nc.compile()
    return nc


_NC_CACHE = None


def _get_nc():
    global _NC_CACHE
    if _NC_CACHE is None:
        _NC_CACHE = build_kernel()
    return _NC_CACHE


def _shard_inputs(inputs):
    bf = ml_dtypes.bfloat16
    x = np.asarray(inputs["input"], dtype=np.float32)
    wv = np.asarray(inputs["wv"], dtype=np.float32)
    bv = np.asarray(inputs["bv"], dtype=np.float32)
    fc_w = np.asarray(inputs["fc_w"], dtype=np.float32)
    fc_b = np.asarray(inputs["fc_b"], dtype=np.float32)
    ln_g = np.asarray(inputs["ln_g"], dtype=np.float32)
    ln_b = np.asarray(inputs["ln_b"], dtype=np.float32)

    # blocked layouts (see build_kernel): each 1MB DMA is contiguous
    wv_bl = np.ascontiguousarray(
        wv.reshape(KD, P, KO // 4, D).transpose(2, 1, 0, 3)).astype(bf)
    fc_bl = np.ascontiguousarray(
        fc_w.reshape(KO // 4, 4, P, D).transpose(0, 2, 1, 3)).astype(bf)
    bvT = np.ascontiguousarray(bv.reshape(KO, P).T)          # [128, 32]
    fcb = np.ascontiguousarray(fc_b[None, :])
    g = np.ascontiguousarray(ln_g[None, :])
    b = np.ascontiguousarray(ln_b[None, :])

    in_maps = []
    for i in range(N_CORES):
        x_bl = np.ascontiguousarray(
            x[i].reshape(2, 4, P, D).transpose(0, 2, 1, 3)).astype(bf)
        xT_bl = np.ascontiguousarray(x[i].T.reshape(KD, P, L)).astype(bf)
        in_maps.append({
            "x": x_bl,
            "xT": xT_bl,
            "wv": wv_bl,
            "bvT": bvT,
            "fc_w": fc_bl,
            "fc_b": fcb,
            "ln_g": g,
            "ln_b": b,
        })
    return in_maps


def kernel(**inputs) -> np.ndarray:
    nc = _get_nc()
    in_maps = _shard_inputs(inputs)
    res = run_bass_kernel_spmd(nc, in_maps, core_ids=list(range(N_CORES)))
    out = np.stack([res.results[i]["out"] for i in range(N_CORES)], axis=0)
    return out.astype(np.float32)


def _install_ntff_hook_shim():
    """Bridge trn_boot's ctypes NTFF profiler into antenv.axon_hooks,
    which bass_utils imports when trace=True under axon."""
    import sys
    import types
    try:
        from antenv.axon_hooks import get_axon_ntff_profile_hook  # noqa: F401
        return
    except ImportError:
        pass
    try:
        from trn_agent_boot.trn_boot import _ntff_profile_via_ctypes
        hook = _ntff_profile_via_ctypes("/opt/axon/libaxon_pjrt.so")
    except Exception:
        hook = None
    mod = types.ModuleType("antenv.axon_hooks")
    state = {"hook": hook}
    mod.get_axon_ntff_profile_hook = lambda: state["hook"]
    mod.set_axon_ntff_profile_hook = lambda h: state.update(hook=h)
    sys.modules["antenv.axon_hooks"] = mod
    import antenv
    antenv.axon_hooks = mod


def kernel_profiled(inputs, trace_cores=None):
    """Like kernel() but with trace=True; returns (out, BassKernelResults)."""
    _install_ntff_hook_shim()
    nc = _get_nc()
    in_maps = _shard_inputs(inputs)
    res = run_bass_kernel_spmd(
        nc, in_maps, core_ids=list(range(N_CORES)), trace=True,
        trace_cores=trace_cores if trace_cores is not None else [0])
    out = np.stack([res.results[i]["out"] for i in range(N_CORES)], axis=0)
    return out.astype(np.float32), res


if __name__ == "__main__":
    import sys
    if "--sim" in sys.argv:
        # quick single-core CoreSim check against the collapsed math
        from concourse.bass_interp import CoreSim
        rng = np.random.default_rng(0)
        x = rng.standard_normal((B, L, D), dtype=np.float32)
        wv = rng.standard_normal((D, HD), dtype=np.float32) * 0.04
        bv = rng.standard_normal(HD, dtype=np.float32) * 0.04
        fc_w = rng.standard_normal((HD, D), dtype=np.float32) * 0.015
        fc_b = rng.standard_normal(D, dtype=np.float32) * 0.015
        g = rng.standard_normal(D, dtype=np.float32) * 0.3 + 1.0
        b = rng.standard_normal(D, dtype=np.float32) * 0.1
        inputs = dict(input=x, wv=wv, bv=bv, fc_w=fc_w, fc_b=fc_b,
                      ln_g=g, ln_b=b)

        nc = _get_nc()
        in_maps = _shard_inputs(inputs)
        sim = CoreSim(nc, trace=False)
        for k, v in in_maps[0].items():
            sim.tensor(k)[:] = v
        sim.simulate()
        got = np.array(sim.tensor("out"))

        xsum = x[0].sum(0)
        z = (xsum @ wv + L * bv) @ fc_w + fc_b
        y = x[0] + z[None, :]
        mu = y.mean(-1, keepdims=True)
        var = y.var(-1, keepdims=True)
        want = (y - mu) / np.sqrt(var + EPS) * g + b
        err = np.abs(got - want).max() / np.abs(want).max()
        print("sim absmax rel err:", err)
        assert err < 2e-2, err
        print("SIM PASS")


# revision 46
# speedup vs baseline: 1.0155x; 1.0155x over previous
"""Trainium2 Bass kernel for nn_MultiHeadAttention_26482768347194.

Key algebraic fact: the reference applies softmax over a size-1 trailing
axis, so the attention score matrix is exactly all-ones.  The whole module
collapses (exactly, in real arithmetic) to

    xsum[b]   = sum_l x[b, l, :]                        # (D,)
    t[b]      = xsum[b] @ wv + L * bv                   # (H*D,)
    z[b]      = t[b] @ fc_w + fc_b                      # (D,)
    y[b,l,:]  = x[b,l,:] + z[b]
    out       = LayerNorm(y) * ln_g + ln_b              # over last dim

q/k/tanh/score inputs are mathematically dead.

Sharding: pure data-parallel over batch, one batch element per core,
weights replicated.  Cross-core collectives measured ~70us under this
runtime (launch-skew barrier), so each core runs fully independently:
  1. xsum.T from its x shard (x-stationary PE matmuls against ones),
  2. streamed by 512-wide hd blocks: t.T columns = wv.T @ xsum.T
     (wv chunks stationary, per-column rotating PSUM banks), ACT moves
     each column to SBUF adding L*bv, and matmul2 accumulates
     z = t @ fc_w into a single PSUM bank,
  3. layernorm decomposed so z only enters through cheap terms: with
     zc = (z + fc_b) - mean(z + fc_b),
       var_y[t] = var_x[t] + (2/D) (x_t . zc) + mean(zc^2)
       out[t]   = ((x_t - mean_x[t]) g + zc g) rstd[t] + ln_b
     so per-token x stats and (x-mean_x)*g run on the DVE DURING the
     weight stream, the x.zc dots run on the then-idle PE (against a
     host-supplied x.T), and the post-z critical path is only two DVE
     passes per token tile.

PE matmuls run in bf16 (fp32 is 2-4x slower on the PE and doubles DMA);
statistics and the residual path stay fp32.  End-to-end absmax relative
error vs the fp32 reference: ~3e-3 (bf16 input/weight rounding, well
inside the scale-relative gate).  Measured ~62-66us on hardware.

This file is self-contained: shapes are hardcoded, no sibling imports.
"""

from contextlib import ExitStack

import numpy as np
import ml_dtypes

import concourse.bass as bass
import concourse.bacc as bacc
import concourse.mybir as mybir
import concourse.tile as tile
from concourse.bass_utils import run_bass_kernel_spmd
from concourse.bass import _add_dep_helper

B, L, D, H = 8, 1024, 512, 8
HD = H * D          # 4096
P = 128             # partitions
NT = L // P         # 8 token tiles per core
KD = D // P         # 4 contraction chunks over d
KO = HD // P        # 32 contraction chunks over h*d
EPS = 1e-5
N_CORES = 8

F32 = mybir.dt.float32
BF16 = mybir.dt.bfloat16
AF = mybir.ActivationFunctionType
ALU = mybir.AluOpType

def build_kernel():
    nc = bacc.Bacc("TRN2", target_bir_lowering=False, debug=False,
                   num_devices=N_CORES)

    # x / wv / fc arrive pre-blocked from the host so that every big DMA
    # reads a fully contiguous region:
    #   x[u, p, t, d]  = x_orig[(u*4 + t)*128 + p, d]          (2 x 1MB)
    #   wv[j, p, k, c] = wv_orig[k*128 + p, j*512 + c]         (8 x 1MB)
    #   fc[j, p, q, d] = fc_orig[(j*4 + q)*128 + p, d]         (8 x 1MB)
    #   xT[c, p, l]    = x_orig[l, c*128 + p]                  (4 x 256KB)
    x_d = nc.dram_tensor("x", [2, P, 4, D], BF16, kind="ExternalInput")
    xT_d = nc.dram_tensor("xT", [KD, P, L], BF16, kind="ExternalInput")
    wv_d = nc.dram_tensor("wv", [KO // 4, P, KD, D], BF16, kind="ExternalInput")
    bvT_d = nc.dram_tensor("bvT", [P, KO], F32, kind="ExternalInput")
    fc_d = nc.dram_tensor("fc_w", [KO // 4, P, 4, D], BF16, kind="ExternalInput")
    fcb_d = nc.dram_tensor("fc_b", [1, D], F32, kind="ExternalInput")
    g_d = nc.dram_tensor("ln_g", [1, D], F32, kind="ExternalInput")
    b_d = nc.dram_tensor("ln_b", [1, D], F32, kind="ExternalInput")
    out_d = nc.dram_tensor("out", [L, D], F32, kind="ExternalOutput")

    x_v = x_d.ap()                                               # [2, P, 4, D]
    wv_v = wv_d.ap()                                             # [8, P, KD, D]
    fc_v = fc_d.ap()                                             # [8, P, 4, D]
    out_v = out_d.ap().rearrange("(t p) d -> t p d", p=P)        # [NT, P, D]

    with tile.TileContext(nc) as tc, ExitStack() as ctx:
        consts = ctx.enter_context(tc.tile_pool(name="consts", bufs=1))
        work = ctx.enter_context(tc.tile_pool(name="work", bufs=3))
        ypool = ctx.enter_context(tc.tile_pool(name="ypool", bufs=8))
        psum = ctx.enter_context(
            tc.tile_pool(name="psum", bufs=1, space=bass.MemorySpace.PSUM))

        # ---- constants -------------------------------------------------
        ones_col = consts.tile([P, 1], BF16)     # token-sum matmul rhs
        nc.gpsimd.memset(ones_col[:], 1.0)
        ones2 = consts.tile([2, P], F32)         # z+fc_b broadcast lhsT
        nc.gpsimd.memset(ones2[:], 1.0)
        eps_t = consts.tile([P, 1], F32)
        nc.gpsimd.memset(eps_t[:], EPS)
        ident_bf = consts.tile([1, 1], BF16)   # 1x1 identity for transposes
        nc.gpsimd.memset(ident_bf[:], 1.0)

        bvT_t = consts.tile([P, KO], F32)
        nc.sync.dma_start(bvT_t[:], bvT_d.ap())
        bv1024 = consts.tile([P, KO], F32)
        nc.scalar.mul(bv1024[:], bvT_t[:], float(L))

        fcb_t = consts.tile([1, D], F32)
        nc.sync.dma_start(fcb_t[:], fcb_d.ap())
        g_t = consts.tile([1, D], F32)
        nc.sync.dma_start(g_t[:], g_d.ap())
        b_t = consts.tile([1, D], F32)
        nc.sync.dma_start(b_t[:], b_d.ap())

        # broadcast ln_g / ln_b to all 128 partitions via a K=1 matmul
        g_bc = consts.tile([P, D], F32)
        b_bc = consts.tile([P, D], F32)
        for src, dst in ((g_t, g_bc), (b_t, b_bc)):
            pb = psum.tile([P, D], F32, tag="bcast")
            nc.tensor.matmul(pb[:], ones2[0:1, :], src[:], start=True, stop=True)
            nc.vector.tensor_copy(dst[:], pb[:])

        # ---- load x and x.T (stay resident in SBUF) --------------------
        x_t = consts.tile([P, NT, D], BF16)
        last_x_dma = None
        for u in range(2):
            for h in range(2):
                last_x_dma = nc.sync.dma_start(
                    x_t[:, u * 4 + h * 2:u * 4 + h * 2 + 2, :],
                    x_v[u][:, h * 2:(h + 1) * 2, :])

        # ---- phase A: xsumT[d] = sum over tokens of x ------------------
        # lhsT = x tile slice [128 tok, 128 d], rhs = ones -> psum [128 d, 1]
        ps_xs = psum.tile([P, KD], F32, tag="bcast")
        for c in range(KD):
            for t in range(NT):
                nc.tensor.matmul(
                    ps_xs[:, c:c + 1],
                    x_t[:, t, c * P:(c + 1) * P],
                    ones_col[:],
                    start=(t == 0), stop=(t == NT - 1))
        xsT = consts.tile([P, KD], BF16)
        nc.vector.tensor_copy(xsT[:], ps_xs[:])

        # ---- early layernorm stats on x (overlaps the z phase) ---------
        # Since z is constant across tokens:
        #   y = x + z',  z' = z + fc_b,  mz = mean(z'),  zc = z' - mz
        #   mean_y[t] = mean_x[t] + mz
        #   var_y[t]  = var_x[t] + (2/D) * (x_t . zc) + mean(zc^2)
        #   out[t]    = ((x_t - mean_x[t])*g + zc*g) * rstd[t] + b
        # so per-token x statistics and (x - mean_x)*g run during the z
        # phase on the otherwise-idle DVE, and the x.zc dots run on the
        # otherwise-idle PE afterwards.
        inv_d = 1.0 / D
        varx8 = consts.tile([P, NT], F32)
        xg_tiles = []
        for t in range(NT):
            s6 = work.tile([P, 6], F32, tag="s6")
            nc.vector.bn_stats(s6[:], x_t[:, t, :])
            mv = work.tile([P, 2], F32, tag="mv")
            nc.vector.bn_aggr(mv[:], s6[:])
            nc.vector.tensor_copy(varx8[:, t:t + 1], mv[:, 1:2])
            negmx = work.tile([P, 1], F32, tag="negmx")
            nc.vector.tensor_scalar_mul(negmx[:], mv[:, 0:1], -1.0)
            xg = ypool.tile([P, D], F32, tag="xg")
            nc.vector.scalar_tensor_tensor(
                xg[:], x_t[:, t, :], negmx[:], g_bc[:],
                op0=ALU.add, op1=ALU.mult)
            xg_tiles.append(xg)

        # ---- phase B+C interleaved, streamed by 512-wide hd blocks -----
        # block j: DMA wv cols [4j*128,(4j+4)*128) + fc rows likewise, then
        #   tT col o = sum_kd wv[:, kd, oc*128:..].T @ xsT[:, kd]   (psum)
        #   ACT copies col to SBUF adding L*bv, matmul2 accumulates z.
        tT_sb = consts.tile([P, KO], BF16)
        ps_z = psum.tile([1, D], F32, tag="z")
        for j in range(KO // 4):
            wv_bt = work.tile([P, KD, D], BF16, tag="wvb", bufs=8)
            i_wv = nc.sync.dma_start(wv_bt[:], wv_v[j])
            _add_dep_helper(i_wv.ins, last_x_dma.ins, sync=False,
                            reason="x shard streams before the weight blocks")
            fc_bt = work.tile([P, 4, D], BF16, tag="fcb", bufs=8)
            i_fc = nc.sync.dma_start(fc_bt[:], fc_v[j])
            _add_dep_helper(i_fc.ins, last_x_dma.ins, sync=False,
                            reason="x shard streams before the weight blocks")
            last_w_dma = i_fc
            for oc in range(4):
                o = 4 * j + oc
                # per-column psum tile (rotating banks) so the ACT read of
                # column o doesn't serialize the PE writes of column o+1
                ps_col = psum.tile([P, 1], F32, tag="small", bufs=4)
                for kd in range(KD):
                    nc.tensor.matmul(
                        ps_col[:],
                        wv_bt[:, kd, oc * P:(oc + 1) * P],
                        xsT[:, kd:kd + 1],
                        start=(kd == 0), stop=(kd == KD - 1))
                nc.scalar.activation(tT_sb[:, o:o + 1], ps_col[:],
                                     AF.Identity, bias=bv1024[:, o:o + 1],
                                     scale=1.0)
                nc.tensor.matmul(
                    ps_z[:],
                    tT_sb[:, o:o + 1],
                    fc_bt[:, oc, :],
                    start=(o == 0), stop=(o == KO - 1),
                    skip_group_check=True)

        xT_t = consts.tile([P, KD, L], BF16)
        for c in range(KD):
            i_xt = nc.sync.dma_start(xT_t[:, c, :], xT_d.ap()[c])
            _add_dep_helper(i_xt.ins, last_w_dma.ins, sync=False,
                            reason="xT load deferred behind the weight stream")

        # ---- phase D: z tail -> zc, zg broadcast, variance pieces ------
        zrow = consts.tile([1, D], F32)
        zsum = consts.tile([1, 1], F32)
        nc.vector.scalar_tensor_tensor(
            zrow[:], fcb_t[:], 1.0, ps_z[:], op0=ALU.mult, op1=ALU.add,
            accum_out=zsum[:])
        negmz = consts.tile([1, 1], F32)
        nc.scalar.mul(negmz[:], zsum[:], -inv_d)
        zc = consts.tile([1, D], F32)
        nc.scalar.activation(zc[:], zrow[:], AF.Identity, bias=negmz[:])
        zc_bf = consts.tile([1, D], BF16)
        nc.vector.tensor_copy(zc_bf[:], zc[:])

        # zcT (for the PE dot products)
        zcT = consts.tile([P, KD], BF16)
        for c in range(KD):
            pzt = psum.tile([P, 1], BF16, tag="small", bufs=4)
            nc.tensor.transpose(pzt[:], zc_bf[0:1, c * P:(c + 1) * P],
                                ident_bf[0:1, 0:1])
            nc.vector.tensor_copy(zcT[:, c:c + 1], pzt[:])

        # zg = zc * g, broadcast to 128 partitions
        zg = consts.tile([1, D], F32)
        nc.vector.scalar_tensor_tensor(
            zg[:], zc[:], 1.0, g_t[:], op0=ALU.mult, op1=ALU.mult)
        ps_zg = psum.tile([P, D], F32, tag="bcast")
        nc.tensor.matmul(ps_zg[:], ones2[0:1, :], zg[:], start=True, stop=True)
        zg_bc = consts.tile([P, D], F32)
        nc.vector.tensor_copy(zg_bc[:], ps_zg[:])

        # mean(zc^2) broadcast into a per-partition bias (+ eps)
        zcsq = consts.tile([1, D], F32)
        nc.vector.tensor_mul(zcsq[:], zc[:], zc[:])
        ezsum = consts.tile([1, 1], F32)
        nc.vector.tensor_reduce(ezsum[:], zcsq[:], axis=mybir.AxisListType.X,
                                op=ALU.add)
        ps_ez = psum.tile([P, 1], F32, tag="small", bufs=4)
        nc.tensor.matmul(ps_ez[:], ones2[0:1, :], ezsum[:],
                         start=True, stop=True)
        bias8 = consts.tile([P, 1], F32)
        nc.scalar.activation(bias8[:], ps_ez[:], AF.Identity,
                             bias=eps_t[:], scale=inv_d)

        # ---- phase E: per-token dots on the PE, then two DVE passes ----
        for t in range(NT):
            pd = psum.tile([P, 1], F32, tag="small", bufs=4)
            for c in range(KD):
                nc.tensor.matmul(pd[:], xT_t[:, c, t * P:(t + 1) * P],
                                 zcT[:, c:c + 1],
                                 start=(c == 0), stop=(c == KD - 1))
            var_t = work.tile([P, 1], F32, tag="var_t", bufs=4)
            nc.vector.scalar_tensor_tensor(
                var_t[:], pd[:], 2.0 * inv_d, varx8[:, t:t + 1],
                op0=ALU.mult, op1=ALU.add)
            std_t = work.tile([P, 1], F32, tag="std_t", bufs=4)
            nc.scalar.activation(std_t[:], var_t[:], AF.Sqrt, bias=bias8[:])
            rstd_t = work.tile([P, 1], F32, tag="rstd_t", bufs=4)
            nc.vector.reciprocal(rstd_t[:], std_t[:])

            yg = work.tile([P, D], F32, tag="yg", bufs=4)
            nc.vector.tensor_add(yg[:], xg_tiles[t][:], zg_bc[:])
            o2 = work.tile([P, D], F32, tag="o2", bufs=4)
            nc.vector.scalar_tensor_tensor(
                o2[:], yg[:], rstd_t[:], b_bc[:],
                op0=ALU.mult, op1=ALU.add)
            nc.sync.dma_start(out_v[t], o2[:])

    nc.compile()
    return nc


_NC_CACHE = None


def _get_nc():
    global _NC_CACHE
    if _NC_CACHE is None:
        _NC_CACHE = build_kernel()
    return _NC_CACHE


def _shard_inputs(inputs):
    bf = ml_dtypes.bfloat16
    x = np.asarray(inputs["input"], dtype=np.float32)
    wv = np.asarray(inputs["wv"], dtype=np.float32)
    bv = np.asarray(inputs["bv"], dtype=np.float32)
    fc_w = np.asarray(inputs["fc_w"], dtype=np.float32)
    fc_b = np.asarray(inputs["fc_b"], dtype=np.float32)
    ln_g = np.asarray(inputs["ln_g"], dtype=np.float32)
    ln_b = np.asarray(inputs["ln_b"], dtype=np.float32)

    # blocked layouts (see build_kernel): each 1MB DMA is contiguous
    wv_bl = np.ascontiguousarray(
        wv.reshape(KD, P, KO // 4, D).transpose(2, 1, 0, 3)).astype(bf)
    fc_bl = np.ascontiguousarray(
        fc_w.reshape(KO // 4, 4, P, D).transpose(0, 2, 1, 3)).astype(bf)
    bvT = np.ascontiguousarray(bv.reshape(KO, P).T)          # [128, 32]
    fcb = np.ascontiguousarray(fc_b[None, :])
    g = np.ascontiguousarray(ln_g[None, :])
    b = np.ascontiguousarray(ln_b[None, :])

    in_maps = []
    for i in range(N_CORES):
        x_bl = np.ascontiguousarray(
            x[i].reshape(2, 4, P, D).transpose(0, 2, 1, 3)).astype(bf)
        xT_bl = np.ascontiguousarray(x[i].T.reshape(KD, P, L)).astype(bf)
        in_maps.append({
            "x": x_bl,
            "xT": xT_bl,
            "wv": wv_bl,
            "bvT": bvT,
            "fc_w": fc_bl,
            "fc_b": fcb,
            "ln_g": g,
            "ln_b": b,
        })
    return in_maps


def kernel(**inputs) -> np.ndarray:
    nc = _get_nc()
    in_maps = _shard_inputs(inputs)
    res = run_bass_kernel_spmd(nc, in_maps, core_ids=list(range(N_CORES)))
    out = np.stack([res.results[i]["out"] for i in range(N_CORES)], axis=0)
    return out.astype(np.float32)


def _install_ntff_hook_shim():
    """Bridge trn_boot's ctypes NTFF profiler into antenv.axon_hooks,
    which bass_utils imports when trace=True under axon."""
    import sys
    import types
    try:
        from antenv.axon_hooks import get_axon_ntff_profile_hook  # noqa: F401
        return
    except ImportError:
        pass
    try:
        from trn_agent_boot.trn_boot import _ntff_profile_via_ctypes
        hook = _ntff_profile_via_ctypes("/opt/axon/libaxon_pjrt.so")
    except Exception:
        hook = None
    mod = types.ModuleType("antenv.axon_hooks")
    state = {"hook": hook}
    mod.get_axon_ntff_profile_hook = lambda: state["hook"]
    mod.set_axon_ntff_profile_hook = lambda h: state.update(hook=h)
    sys.modules["antenv.axon_hooks"] = mod
    import antenv
    antenv.axon_hooks = mod


def kernel_profiled(inputs, trace_cores=None):
    """Like kernel() but with trace=True; returns (out, BassKernelResults)."""
    _install_ntff_hook_shim()
    nc = _get_nc()
    in_maps = _shard_inputs(inputs)
    res = run_bass_kernel_spmd(
        nc, in_maps, core_ids=list(range(N_CORES)), trace=True,
        trace_cores=trace_cores if trace_cores is not None else [0])
    out = np.stack([res.results[i]["out"] for i in range(N_CORES)], axis=0)
    return out.astype(np.float32), res


if __name__ == "__main__":
    import sys
    if "--sim" in sys.argv:
        # quick single-core CoreSim check against the collapsed math
        from concourse.bass_interp import CoreSim
        rng = np.random.default_rng(0)
        x = rng.standard_normal((B, L, D), dtype=np.float32)
        wv = rng.standard_normal((D, HD), dtype=np.float32) * 0.04
        bv = rng.standard_normal(HD, dtype=np.float32) * 0.04
        fc_w = rng.standard_normal((HD, D), dtype=np.float32) * 0.015
        fc_b = rng.standard_normal(D, dtype=np.float32) * 0.015
        g = rng.standard_normal(D, dtype=np.float32) * 0.3 + 1.0
        b = rng.standard_normal(D, dtype=np.float32) * 0.1
        inputs = dict(input=x, wv=wv, bv=bv, fc_w=fc_w, fc_b=fc_b,
                      ln_g=g, ln_b=b)

        nc = _get_nc()
        in_maps = _shard_inputs(inputs)
        sim = CoreSim(nc, trace=False)
        for k, v in in_maps[0].items():
            sim.tensor(k)[:] = v
        sim.simulate()
        got = np.array(sim.tensor("out"))

        xsum = x[0].sum(0)
        z = (xsum @ wv + L * bv) @ fc_w + fc_b
        y = x[0] + z[None, :]
        mu = y.mean(-1, keepdims=True)
        var = y.var(-1, keepdims=True)
        want = (y - mu) / np.sqrt(var + EPS) * g + b
        err = np.abs(got - want).max() / np.abs(want).max()
        print("sim absmax rel err:", err)
        assert err < 2e-2, err
        print("SIM PASS")
